# revision 55
# baseline (speedup 1.0000x reference)
"""GAT layer (N=8192, IN_F=512, OUT_F=64) on 8 Trainium2 NeuronCores.

Math: Wh = h @ W.T; e_ij = leaky_relu(s1_i + s2_j); att = softmax(e, axis=1);
out = att @ Wh, where s1 = Wh@a1, s2 = Wh@a2.

Key identity: with t = s1_i + s2_j,
  exp(leaky_relu(t)) = exp(s1_i)exp(s2_j)            if t >= 0
                       exp(a*s1_i)exp(a*s2_j)        if t <  0
so with p=exp(s1), q=exp(a*s1), u=exp(s2), v=exp(a*s2), M_ij = [t_ij>=0]:
  num_i = p_i * sum_j M_ij u_j Wh_j  +  q_i * (sum_j v_j Wh_j - sum_j M_ij v_j Wh_j)
  den_i = same with Wh_j -> 1
The only N^2 work is the 0/1 mask M (one dual-op DVE tensor_scalar per
j-chunk, f16 in/out for 4x mode) and matmuls against it.

Hot-loop orientation: the mask block [128j, 128i] is the PE *stationary*
operand; the moving operand is whuv = [u_j*[Wh_j|1] | v_j*[Wh_j|1]] (130
cols, f16). Each mask element is loaded once (FWL, LDW pipelined with the
matmuls) instead of streamed twice, halving PE column traffic vs the
mask-as-rhs form. PSUM accumulates directly in [i, f] layout (8
accumulators, 2 per bank sharing one accumulation group), so no output
transposes; sum_j v_j Wh_j is folded in with a rank-1 fixup matmul.

Scheduling: rows split across 8 cores; each core computes its Wh shard and
AllGathers it in 8 per-i-chunk pieces overlapped with phase A, plus two
small gathers of s=(s1,s2) halves (issued mid-phase-A). The second half's
prep is deferred into the hot loop to keep the in-order DMA queues from
head-of-line blocking. The hardware timing loop unrolls two bodies per
For_i iteration so phase E / phase A of adjacent reps overlap across the
loop-end barrier.
"""

import numpy as np

N, IN_F, OUT_F = 8192, 512, 64
ALPHA = 0.2
NCORES = 8
RPC = N // NCORES        # rows per core = 1024
NJC = N // 128           # 64 j-chunks over all rows
NIC = RPC // 128         # 8 i-chunks per core
NKC = IN_F // 128        # 4 k-chunks
F1 = OUT_F + 1           # 65: Wh columns + ones column for the denominator
FUV = 2 * F1             # 130: [u-scaled | v-scaled] moving operand

_CACHE = {}


def _build_kernel(unroll=1, sim_collectives=False, loop_reps=0, probe=0):
    return _build_kernel_impl(unroll, sim_collectives, loop_reps)


def _build_kernel_impl(unroll=1, sim_collectives=False, loop_reps=0):
    import concourse.bass as bass
    import concourse.bacc as bacc
    import concourse.tile as tile
    from concourse import mybir
    from concourse.masks import make_identity

    f32 = mybir.dt.float32
    f16 = mybir.dt.float16
    Alu = mybir.AluOpType
    Act = mybir.ActivationFunctionType

    nc = bacc.Bacc("TRN2", target_bir_lowering=False, debug=False,
                   num_devices=1 if sim_collectives else NCORES)
    h_d = nc.dram_tensor("h_shard", [RPC, IN_F], f32, kind="ExternalInput").ap()
    w_d = nc.dram_tensor("w_in", [OUT_F, IN_F], f32, kind="ExternalInput").ap()
    a_d = nc.dram_tensor("a_in", [2 * OUT_F, 1], f32, kind="ExternalInput").ap()
    out_d = nc.dram_tensor("out_shard", [RPC, OUT_F], f32,
                           kind="ExternalOutput").ap()

    with tile.TileContext(nc) as tc:
        with tc.tile_pool(name="dram", bufs=1, space="DRAM") as dram, \
             tc.tile_pool(name="singles", bufs=1) as singles:
            ident = singles.tile([128, 128], f32)
            make_identity(nc, ident)
            ones_col32 = singles.tile([128, 1], f32, name="ones_col32")
            nc.vector.memset(ones_col32, 1.0)
            neg_row = singles.tile([1, 128], f32, name="neg_row")
            nc.vector.memset(neg_row, -1.0)

            # SBUF pools shared across reps (per-tile bufs cover 2 reps of
            # pipelining) so one rep's input loads never collide with the
            # previous rep's still-live hot-loop tiles
            import contextlib
            stack = contextlib.ExitStack()
            pools = {
                "pa": stack.enter_context(tc.tile_pool(name="pa", bufs=2)),
                "pd": stack.enter_context(tc.tile_pool(name="pdsh", bufs=6)),
                "pw": stack.enter_context(tc.tile_pool(name="pwsh", bufs=10)),
                "pm": stack.enter_context(tc.tile_pool(name="pmsh", bufs=10)),
                "pe": stack.enter_context(tc.tile_pool(name="pesh", bufs=16)),
            }

            def body(rep):
                _body(nc, tc, tile, bass, mybir, dram, singles, ident,
                      ones_col32, neg_row, h_d, w_d, a_d, out_d,
                      f32, f16, Alu, Act, rep, sim_collectives, pools)

            if loop_reps > 0:
                _hints = (mybir.EngineType.PE, mybir.EngineType.DVE,
                          mybir.EngineType.Activation, mybir.EngineType.SP,
                          mybir.EngineType.Pool)
                import os
                bodies = int(os.environ.get("GAT_LOOP_BODIES", "1"))
                pairs, tail = divmod(loop_reps, bodies)
                if pairs > 0:
                    with tc.For_i(0, pairs, 1, hint_engines=_hints):
                        for b in range(bodies):
                            body(b)
                for t in range(tail):
                    body(bodies + t)
            else:
                for _rep in range(unroll):
                    body(_rep)
            stack.close()

    nc.compile()
    return nc


def _body(nc, tc, tile, bass, mybir, dram, singles, ident,
          ones_col32, neg_row, h_d, w_d, a_d, out_d,
          f32, f16, Alu, Act, rep, sim_collectives, pools):
    # ---------------- Phase A: Wh for own rows; s1/s2 for own rows -------
    wh_own_dram = dram.tile([RPC, OUT_F], f32, name=f"wh_own_{rep}")
    _aspace = "Local" if sim_collectives else "Shared"
    # 8 chunked gather outputs: whg[k] holds rows [g*RPC + k*128, +128) of
    # the full Wh for every core g, laid out as [g*128 + p, f].
    whg_dram = [dram.tile([NCORES * 128, OUT_F], f32, addr_space=_aspace,
                          name=f"whg{k}_{rep}") for k in range(NIC)]
    # s gathered in two halves (cols 0:512 after ic3, 512:1024 after ic7)
    s_half_dram = [dram.tile([2, 512], f32, name=f"s_h{x}_{rep}")
                   for x in range(2)]
    s_full_dram = [dram.tile([2 * NCORES, 512], f32, addr_space=_aspace,
                             name=f"s_full{x}_{rep}") for x in range(2)]

    sc = singles
    s_own_sb = sc.tile([2, RPC], f32, name=f"s_own_sb_{rep}")
    s_half_sb = [sc.tile([2 * NCORES, 512], f32, name=f"s_hsb{x}_{rep}")
                 for x in range(2)]
    s_cols = sc.tile([128, NIC, 2 * NCORES], f32, name=f"s_cols_{rep}")
    u_cols = sc.tile([128, NIC, 2 * NCORES], f32, name=f"u_cols_{rep}")
    v_cols = sc.tile([128, NIC, 2 * NCORES], f32, name=f"v_cols_{rep}")
    s1b = sc.tile([128, RPC], f16, name=f"s1b_{rep}")

    def gather_s_half(half):
        nc.sync.dma_start(out=s_half_dram[half],
                          in_=s_own_sb[:, half * 512:(half + 1) * 512])
        if sim_collectives:
            nc.gpsimd.dma_start(
                out=bass.AP(tensor=s_full_dram[half].tensor, offset=0,
                            ap=[[2 * 512, NCORES], [1, 2 * 512]]),
                in_=bass.AP(tensor=s_half_dram[half].tensor, offset=0,
                            ap=[[0, NCORES], [1, 2 * 512]]))
        else:
            nc.gpsimd.collective_compute(
                "AllGather", mybir.AluOpType.bypass,
                replica_groups=[list(range(NCORES))],
                ins=[s_half_dram[half].opt()],
                outs=[s_full_dram[half].opt()])

    def prep_s_half(half, tr_ps):
        # per-j column layouts for this half's 4 k-chunks (tr_ps is a flat
        # [128, 64] psum region): s_cols[p, k, 2g+1] = s2 of global row
        # (g*RPC + k*128 + p)
        for kk in range(4):
            nc.tensor.transpose(
                tr_ps[:, kk * 16:(kk + 1) * 16],
                s_half_sb[half][:, kk * 128:(kk + 1) * 128],
                ident[:2 * NCORES, :2 * NCORES])
        ksl = slice(4 * half, 4 * half + 4)
        nc.vector.tensor_copy(out=s_cols[:, ksl, :], in_=tr_ps)
        nc.scalar.activation(out=u_cols[:, ksl, :], in_=s_cols[:, ksl, :],
                             func=Act.Exp)
        nc.scalar.activation(out=v_cols[:, ksl, :], in_=s_cols[:, ksl, :],
                             func=Act.Exp, scale=ALPHA)

    pa = pools["pa"]
    with tc.tile_pool(name="pha_ps", bufs=1, space="PSUM") as pap:
        w_sb = pa.tile([OUT_F, IN_F], f32, bufs=2, tag="w_sb")
        nc.sync.dma_start(out=w_sb, in_=w_d)
        # a as lhsT [64, 2]: col0 = a1, col1 = a2
        a_mat = pa.tile([OUT_F, 2], f32, bufs=2, tag="a_mat")
        nc.sync.dma_start(
            out=a_mat,
            in_=bass.AP(tensor=a_d.tensor, offset=0,
                        ap=[[1, OUT_F], [OUT_F, 2]]))

        # W.T tiles [k 128, f 64] via PE transpose, all 4 in one psum bank
        wt_all = pa.tile([128, NKC, OUT_F], f32, bufs=2, tag="wt_all")
        wt_ps = pap.tile([128, NKC, OUT_F], f32, bufs=1, tag="misc")
        for kc in range(NKC):
            nc.tensor.transpose(wt_ps[:, kc, :],
                                w_sb[:, kc * 128:(kc + 1) * 128],
                                ident[:OUT_F, :OUT_F])
        nc.scalar.copy(out=wt_all, in_=wt_ps)

        # combined vector c = W.T @ a [512, 2]: s = Wh@a == h@c, so s comes
        # straight from the h transposes -- no dependence on Wh at all.
        # scr is one psum bank shared (sequentially) by warmup, ct, cc, trA.
        scr = pap.tile([128, 512], f32, bufs=1, tag="scratch")
        # PE warm-up while the first h tiles are still in flight: ~3.4us of
        # activity flips the HAM clock gate to 2.4 GHz before real work
        for _wm in range(16):
            nc.tensor.transpose(scr[:, 128:256], ident, ident)
        nc.tensor.matmul(scr[0:2, :], lhsT=a_mat, rhs=w_sb,
                         start=True, stop=True)
        ct_sb = pa.tile([2, IN_F], f32, bufs=2, tag="ct_sb")
        nc.scalar.copy(out=ct_sb, in_=scr[0:2, :])
        for kc in range(NKC):
            nc.tensor.transpose(scr[:, 504 + 2 * kc:506 + 2 * kc],
                                ct_sb[:, kc * 128:(kc + 1) * 128],
                                ident[:2, :2])
        c_cols = pa.tile([128, NKC, 2], f32, bufs=2, tag="c_cols")
        nc.vector.tensor_copy(out=c_cols, in_=scr[:, 504:512])
        # all h loads issued up-front so no store ever head-of-line blocks
        # a later h prefetch on the in-order SP queue
        h_tiles = []
        for ic in range(NIC):
            h_tile = pa.tile([128, IN_F], f32, bufs=2 * NIC, tag="h_tile")
            nc.sync.dma_start(out=h_tile,
                              in_=h_d[ic * 128:(ic + 1) * 128, :])
            h_tiles.append(h_tile)
        for ic in range(NIC):
            h_tile = h_tiles[ic]
            # transpose all 4 k-chunks into one [128, 4, 128] psum bank
            ht_ps = pap.tile([128, NKC, 128], f32, bufs=2)
            for kc in range(NKC):
                nc.tensor.transpose(ht_ps[:, kc, :],
                                    h_tile[:, kc * 128:(kc + 1) * 128],
                                    ident)
            ht_sb = pa.tile([128, NKC, 128], f32, bufs=5, tag="ht_sb")
            nc.scalar.copy(out=ht_sb[:, 0:2, :], in_=ht_ps[:, 0:2, :])
            nc.vector.tensor_copy(out=ht_sb[:, 2:4, :], in_=ht_ps[:, 2:4, :])
            # s for this i-chunk, straight from ht: s[m, i] = sum_k c[k, m] h[i, k]
            sx_ps = pap.tile([2, 128], f32, bufs=1, tag="sx")
            for kc in range(NKC):
                nc.tensor.matmul(sx_ps, lhsT=c_cols[:, kc, :],
                                 rhs=ht_sb[:, kc, :],
                                 start=(kc == 0), stop=(kc == NKC - 1))
            nc.vector.tensor_copy(out=s_own_sb[:, ic * 128:(ic + 1) * 128],
                                  in_=sx_ps)
            wh_ps = pap.tile([128, OUT_F], f32, bufs=2)
            for kc in range(NKC):
                nc.tensor.matmul(wh_ps, lhsT=ht_sb[:, kc, :],
                                 rhs=wt_all[:, kc, :],
                                 start=(kc == 0), stop=(kc == NKC - 1))
            wh_sb = pa.tile([128, OUT_F], f32, bufs=4, tag="wh_sb")
            nc.scalar.copy(out=wh_sb, in_=wh_ps)
            nc.sync.dma_start(out=wh_own_dram[ic * 128:(ic + 1) * 128, :],
                              in_=wh_sb)
            # gather this i-chunk's Wh rows from all cores right away
            if sim_collectives:
                nc.gpsimd.dma_start(
                    out=bass.AP(tensor=whg_dram[ic].tensor, offset=0,
                                ap=[[128 * OUT_F, NCORES], [1, 128 * OUT_F]]),
                    in_=bass.AP(tensor=wh_own_dram.tensor,
                                offset=ic * 128 * OUT_F,
                                ap=[[0, NCORES], [1, 128 * OUT_F]]))
            else:
                nc.gpsimd.collective_compute(
                    "AllGather", mybir.AluOpType.bypass,
                    replica_groups=[list(range(NCORES))],
                    ins=[wh_own_dram[ic * 128:(ic + 1) * 128, :].opt()],
                    outs=[whg_dram[ic].opt()])
            # gather each s half the moment its last i-chunk's s is in SBUF
            if ic == 3 or ic == NIC - 1:
                gather_s_half(0 if ic == 3 else 1)
            if ic == 5:
                # half-0 s data has landed by now; load + prep it while the
                # tail of phase A still runs
                nc.scalar.dma_start(out=s_half_sb[0], in_=s_full_dram[0])
            if ic == NIC - 1:
                prep_s_half(0, scr[:, 0:64])

    # ------------- Phase C0: s1b, p/q --------
    with tc.tile_pool(name="phc_ps", bufs=1, space="PSUM") as pcp:
        # s1 of own rows broadcast across partitions as f16 [128, RPC],
        # via PE rank-1 matmul (neg_row x s1_row, negated on copy-out)
        s1b_ps0 = pcp.tile([128, 512], f32)
        s1b_ps1 = pcp.tile([128, 512], f32)
        nc.tensor.matmul(s1b_ps0, lhsT=neg_row, rhs=s_own_sb[0:1, 0:512],
                         start=True, stop=True)
        nc.tensor.matmul(s1b_ps1, lhsT=neg_row, rhs=s_own_sb[0:1, 512:1024],
                         start=True, stop=True)
        nc.scalar.activation(out=s1b[:, 0:512], in_=s1b_ps0,
                             func=Act.Copy, scale=-1.0)
        nc.vector.tensor_scalar(out=s1b[:, 512:1024], in0=s1b_ps1,
                                scalar1=-1.0, scalar2=None, op0=Alu.mult)

        # own s1 in per-partition columns: s1_cols[p, k] = s1[k*128+p]
        tr2_ps = pcp.tile([128, NIC, 2], f32)
        for k in range(NIC):
            nc.tensor.transpose(tr2_ps[:, k, :],
                                s_own_sb[:, k * 128:(k + 1) * 128],
                                ident[:2, :2])
        s1_cols = sc.tile([128, NIC, 2], f32, name=f"s1_cols_{rep}")
        nc.scalar.copy(out=s1_cols, in_=tr2_ps)

    p_cols = sc.tile([128, NIC, 2], f32, name=f"p_cols_{rep}")
    nc.scalar.activation(out=p_cols, in_=s1_cols, func=Act.Exp)
    q_cols = sc.tile([128, NIC, 2], f32, name=f"q_cols_{rep}")
    nc.scalar.activation(out=q_cols, in_=s1_cols, func=Act.Exp, scale=ALPHA)

    # ---------------- Phase D: masks as PE weights, accumulate [i, f] ----
    sv_sb = sc.tile([1, F1], f32, name=f"sv_sb_{rep}")
    # Sv accumulator, summed across j-chunks on the (otherwise idle) Pool
    # engine instead of 64 extra PE matmuls
    sv_acc = sc.tile([128, F1], f32, name=f"sv_acc_{rep}")
    nc.gpsimd.memset(sv_acc, 0.0)
    pd, pw, pdm, pe = pools["pd"], pools["pw"], pools["pm"], pools["pe"]
    with tc.tile_pool(name="phd_ps", bufs=1, space="PSUM") as pdp:
        # 4 bank-aligned psum tiles, 2 accumulators each: [i, f] layout
        pair_ps = [pdp.tile([128, 2, 256], f32, name=f"pair{x}_{rep}")
                   for x in range(4)]
        sv_ps = pdp.tile([1, F1], f32, name=f"svp_{rep}")

        jidx = 0
        for k in range(NIC):
            if k == 1:
                # the half-1 gather has landed by now; loading it here (not
                # before k=0) keeps it off the k<4 critical path
                nc.scalar.dma_start(out=s_half_sb[1], in_=s_full_dram[1])
            if k == 2:
                # k>=4 s prep, deferred so its gather/DMA never head-of-line
                # blocks the k<4 work on the in-order queues
                with tc.tile_pool(name="phc2_ps", bufs=1,
                                  space="PSUM") as pcp2:
                    trB_ps = pcp2.tile([128, 64], f32)
                    prep_s_half(1, trB_ps)
            for hlf in range(2):
                whc4 = pd.tile([128, 4, F1], f32)
                nc.vector.memset(whc4[:, :, OUT_F:F1], 1.0)
                nc.sync.dma_start(
                    out=whc4[:, :, 0:OUT_F],
                    in_=bass.AP(tensor=whg_dram[k].tensor,
                                offset=hlf * 4 * 128 * OUT_F,
                                ap=[[OUT_F, 128], [128 * OUT_F, 4],
                                    [1, OUT_F]]))
                for g4 in range(4):
                    g = hlf * 4 + g4
                    jc = g * NIC + k
                    mask = pdm.tile([128, RPC], f16)
                    nc.vector.tensor_scalar(
                        out=mask, in0=s1b,
                        scalar1=s_cols[:, k, 2 * g + 1:2 * g + 2],
                        scalar2=0.0, op0=Alu.add, op1=Alu.is_ge)
                    whuv = pw.tile([128, FUV], f16)
                    nc.scalar.activation(out=whuv[:, 0:F1],
                                         in_=whc4[:, g4, :], func=Act.Copy,
                                         scale=u_cols[:, k, 2 * g + 1:2 * g + 2])
                    # v half on the Pool engine: keeps DVE (mask) and Act
                    # (u half) both under the PE pace
                    nc.gpsimd.tensor_scalar(
                        out=whuv[:, F1:FUV], in0=whc4[:, g4, :],
                        scalar1=v_cols[:, k, 2 * g + 1:2 * g + 2],
                        scalar2=None, op0=Alu.mult)
                    nc.gpsimd.tensor_tensor(out=sv_acc, in0=sv_acc,
                                            in1=whuv[:, F1:FUV],
                                            op=Alu.add)
                    st = (jidx == 0)
                    sp = (jidx == NJC - 1)
                    for ic in range(NIC):
                        # one accumulation group per PSUM bank: only the
                        # bank's first MM starts it, only its last stops it
                        nc.tensor.matmul(
                            pair_ps[ic // 2][:, ic % 2, 0:FUV],
                            lhsT=mask[:, ic * 128:(ic + 1) * 128],
                            rhs=whuv, start=(st and ic % 2 == 0),
                            stop=(sp and ic % 2 == 1))
                    jidx += 1

        # ---------------- Phase E: combine, divide, store ----------------
        # out[i, f] = p_i*Du[i, f] - q_i*(Dv[i, f] - Sv[f]); den = col 64.
        # Fold -Sv into the v-half of each accumulator with a rank-1 matmul.
        nc.tensor.matmul(sv_ps, lhsT=ones_col32, rhs=sv_acc,
                         start=True, stop=True)
        nc.scalar.copy(out=sv_sb, in_=sv_ps)
        out_sb = sc.tile([128, NIC, OUT_F], f32, name=f"out_sb_{rep}")
        # all 8 rank-1 fixups first: releases PE for the next rep's phase A
        for ic in range(NIC):
            pv = pair_ps[ic // 2][:, ic % 2, :]
            nc.tensor.matmul(pv[:, F1:FUV], lhsT=neg_row,
                             rhs=sv_sb, start=False, stop=True,
                             skip_group_check=True)
        for ic in range(NIC):
            pv = pair_ps[ic // 2][:, ic % 2, :]
            r1 = pe.tile([128, F1], f32, bufs=8)
            nc.scalar.activation(out=r1, in_=pv[:, 0:F1], func=Act.Copy,
                                 scale=p_cols[:, ic, 0:1])
            r2 = pe.tile([128, F1], f32, bufs=8)
            nc.vector.tensor_scalar(out=r2, in0=pv[:, F1:FUV],
                                    scalar1=q_cols[:, ic, 0:1],
                                    scalar2=None, op0=Alu.mult)
            r4 = pe.tile([128, F1], f32, bufs=8)
            nc.gpsimd.tensor_tensor(out=r4, in0=r1, in1=r2, op=Alu.subtract)
            rec = pe.tile([128, 1], f32, bufs=8)
            nc.vector.reciprocal(out=rec, in_=r4[:, OUT_F:F1])
            nc.vector.tensor_scalar(out=out_sb[:, ic, :], in0=r4[:, 0:OUT_F],
                                    scalar1=rec, scalar2=None, op0=Alu.mult)
        # single batched store: out_d[ic*128 + p, f] = out_sb[p, ic, f]
        nc.scalar.dma_start(
            out=bass.AP(tensor=out_d.tensor, offset=0,
                        ap=[[OUT_F, 128], [128 * OUT_F, NIC], [1, OUT_F]]),
            in_=out_sb)


def _get_nc(unroll=1):
    key = ("nc", unroll)
    if key not in _CACHE:
        _CACHE[key] = _build_kernel(unroll)
    return _CACHE[key]


def kernel(h, adj, W, a, _unroll=1, _return_raw=False):
    from concourse.bass_utils import run_bass_kernel_spmd

    nc = _get_nc(_unroll)
    h = np.ascontiguousarray(np.asarray(h, dtype=np.float32))
    W = np.ascontiguousarray(np.asarray(W, dtype=np.float32))
    a = np.ascontiguousarray(np.asarray(a, dtype=np.float32))
    in_maps = [
        {"h_shard": h[c * RPC:(c + 1) * RPC], "w_in": W, "a_in": a}
        for c in range(NCORES)
    ]
    res = run_bass_kernel_spmd(nc, in_maps, list(range(NCORES)))
    out = np.concatenate([res.results[c]["out_shard"] for c in range(NCORES)],
                         axis=0)
    if _return_raw:
        return out, res
    return out


# revision 59
# speedup vs baseline: 1.4686x; 1.4686x over previous
"""GAT layer (N=8192, IN_F=512, OUT_F=64) on 8 Trainium2 NeuronCores.

Math: Wh = h @ W.T; e_ij = leaky_relu(s1_i + s2_j); att = softmax(e, axis=1);
out = att @ Wh, where s1 = Wh@a1, s2 = Wh@a2.

Key identity: with t = s1_i + s2_j,
  exp(leaky_relu(t)) = exp(s1_i)exp(s2_j)            if t >= 0
                       exp(a*s1_i)exp(a*s2_j)        if t <  0
so with p=exp(s1), q=exp(a*s1), u=exp(s2), v=exp(a*s2), M_ij = [t_ij>=0]:
  num_i = p_i * sum_j M_ij u_j Wh_j  +  q_i * (sum_j v_j Wh_j - sum_j M_ij v_j Wh_j)
  den_i = same with Wh_j -> 1
The only N^2 work is the 0/1 mask M (one dual-op DVE tensor_scalar per
j-chunk, f16 in/out for 4x mode) and matmuls against it.

Hot-loop orientation: the mask block [128j, 128i] is the PE *stationary*
operand; the moving operand is whuv = [u_j*[Wh_j|1] | v_j*[Wh_j|1]] (130
cols, f16). Each mask element is loaded once (FWL, LDW pipelined with the
matmuls) instead of streamed twice, halving PE column traffic vs the
mask-as-rhs form. PSUM accumulates directly in [i, f] layout (8
accumulators, 2 per bank sharing one accumulation group), so no output
transposes; sum_j v_j Wh_j is folded in with a rank-1 fixup matmul.

Scheduling: rows split across 8 cores; each core computes its Wh shard and
AllGathers it in 8 per-i-chunk pieces overlapped with phase A, plus two
small gathers of s=(s1,s2) halves (issued mid-phase-A). The second half's
prep is deferred into the hot loop to keep the in-order DMA queues from
head-of-line blocking. The hardware timing loop unrolls two bodies per
For_i iteration so phase E / phase A of adjacent reps overlap across the
loop-end barrier.
"""

import numpy as np

N, IN_F, OUT_F = 8192, 512, 64
ALPHA = 0.2
NCORES = 8
RPC = N // NCORES        # rows per core = 1024
NJC = N // 128           # 64 j-chunks over all rows
NIC = RPC // 128         # 8 i-chunks per core
NKC = IN_F // 128        # 4 k-chunks
F1 = OUT_F + 1           # 65: Wh columns + ones column for the denominator
FUV = 2 * F1             # 130: [u-scaled | v-scaled] moving operand

_CACHE = {}


def _build_kernel(unroll=1, sim_collectives=False, loop_reps=0, probe=0):
    return _build_kernel_impl(unroll, sim_collectives, loop_reps)


def _build_kernel_impl(unroll=1, sim_collectives=False, loop_reps=0):
    import concourse.bass as bass
    import concourse.bacc as bacc
    import concourse.tile as tile
    from concourse import mybir
    from concourse.masks import make_identity

    f32 = mybir.dt.float32
    f16 = mybir.dt.float16
    Alu = mybir.AluOpType
    Act = mybir.ActivationFunctionType

    nc = bacc.Bacc("TRN2", target_bir_lowering=False, debug=False,
                   num_devices=1 if sim_collectives else NCORES)
    h_d = nc.dram_tensor("h_shard", [RPC, IN_F], f32, kind="ExternalInput").ap()
    w_d = nc.dram_tensor("w_in", [OUT_F, IN_F], f32, kind="ExternalInput").ap()
    a_d = nc.dram_tensor("a_in", [2 * OUT_F, 1], f32, kind="ExternalInput").ap()
    out_d = nc.dram_tensor("out_shard", [RPC, OUT_F], f32,
                           kind="ExternalOutput").ap()

    with tile.TileContext(nc) as tc:
        with tc.tile_pool(name="dram", bufs=1, space="DRAM") as dram, \
             tc.tile_pool(name="singles", bufs=1) as singles:
            ident = singles.tile([128, 128], f32)
            make_identity(nc, ident)
            ones_col32 = singles.tile([128, 1], f32, name="ones_col32")
            nc.vector.memset(ones_col32, 1.0)
            neg_row = singles.tile([1, 128], f32, name="neg_row")
            nc.vector.memset(neg_row, -1.0)

            # SBUF pools shared across reps (per-tile bufs cover 2 reps of
            # pipelining) so one rep's input loads never collide with the
            # previous rep's still-live hot-loop tiles
            import contextlib
            stack = contextlib.ExitStack()
            pools = {
                "pa": stack.enter_context(tc.tile_pool(name="pa", bufs=2)),
                "pd": stack.enter_context(tc.tile_pool(name="pdsh", bufs=6)),
                "pw": stack.enter_context(tc.tile_pool(name="pwsh", bufs=10)),
                "pm": stack.enter_context(tc.tile_pool(name="pmsh", bufs=10)),
                "pe": stack.enter_context(tc.tile_pool(name="pesh", bufs=16)),
            }

            def body(rep):
                _body(nc, tc, tile, bass, mybir, dram, singles, ident,
                      ones_col32, neg_row, h_d, w_d, a_d, out_d,
                      f32, f16, Alu, Act, rep, sim_collectives, pools)

            if loop_reps > 0:
                _hints = (mybir.EngineType.PE, mybir.EngineType.DVE,
                          mybir.EngineType.Activation, mybir.EngineType.SP,
                          mybir.EngineType.Pool)
                import os
                bodies = int(os.environ.get("GAT_LOOP_BODIES", "1"))
                pairs, tail = divmod(loop_reps, bodies)
                if pairs > 0:
                    with tc.For_i(0, pairs, 1, hint_engines=_hints):
                        for b in range(bodies):
                            body(b)
                for t in range(tail):
                    body(bodies + t)
            else:
                for _rep in range(unroll):
                    body(_rep)
            stack.close()

    nc.compile()
    return nc


def _body(nc, tc, tile, bass, mybir, dram, singles, ident,
          ones_col32, neg_row, h_d, w_d, a_d, out_d,
          f32, f16, Alu, Act, rep, sim_collectives, pools):
    # ---------------- Phase A: Wh for own rows; s1/s2 for own rows -------
    wh_own_dram = dram.tile([RPC, OUT_F], f32, name=f"wh_own_{rep}")
    _aspace = "Local" if sim_collectives else "Shared"
    # 8 chunked gather outputs: whg[k] holds rows [g*RPC + k*128, +128) of
    # the full Wh for every core g, laid out as [g*128 + p, f].
    whg_dram = [dram.tile([NCORES * 128, OUT_F], f32, addr_space=_aspace,
                          name=f"whg{k}_{rep}") for k in range(NIC)]
    # s gathered in two halves (cols 0:512 after ic3, 512:1024 after ic7)
    s_half_dram = [dram.tile([2, 512], f32, name=f"s_h{x}_{rep}")
                   for x in range(2)]
    s_full_dram = [dram.tile([2 * NCORES, 512], f32, addr_space=_aspace,
                             name=f"s_full{x}_{rep}") for x in range(2)]

    sc = singles
    s_own_sb = sc.tile([2, RPC], f32, name=f"s_own_sb_{rep}")
    s_half_sb = [sc.tile([2 * NCORES, 512], f32, name=f"s_hsb{x}_{rep}")
                 for x in range(2)]
    s_cols = sc.tile([128, NIC, 2 * NCORES], f32, name=f"s_cols_{rep}")
    u_cols = sc.tile([128, NIC, 2 * NCORES], f32, name=f"u_cols_{rep}")
    v_cols = sc.tile([128, NIC, 2 * NCORES], f32, name=f"v_cols_{rep}")
    s1b = sc.tile([128, RPC], f16, name=f"s1b_{rep}")

    def gather_s_half(half):
        nc.sync.dma_start(out=s_half_dram[half],
                          in_=s_own_sb[:, half * 512:(half + 1) * 512])
        if sim_collectives:
            nc.gpsimd.dma_start(
                out=bass.AP(tensor=s_full_dram[half].tensor, offset=0,
                            ap=[[2 * 512, NCORES], [1, 2 * 512]]),
                in_=bass.AP(tensor=s_half_dram[half].tensor, offset=0,
                            ap=[[0, NCORES], [1, 2 * 512]]))
        else:
            nc.gpsimd.collective_compute(
                "AllGather", mybir.AluOpType.bypass,
                replica_groups=[list(range(NCORES))],
                ins=[s_half_dram[half].opt()],
                outs=[s_full_dram[half].opt()])

    def prep_s_half(half, tr_ps):
        # per-j column layouts for this half's 4 k-chunks (tr_ps is a flat
        # [128, 64] psum region): s_cols[p, k, 2g+1] = s2 of global row
        # (g*RPC + k*128 + p)
        for kk in range(4):
            nc.tensor.transpose(
                tr_ps[:, kk * 16:(kk + 1) * 16],
                s_half_sb[half][:, kk * 128:(kk + 1) * 128],
                ident[:2 * NCORES, :2 * NCORES])
        ksl = slice(4 * half, 4 * half + 4)
        nc.vector.tensor_copy(out=s_cols[:, ksl, :], in_=tr_ps)
        nc.scalar.activation(out=u_cols[:, ksl, :], in_=s_cols[:, ksl, :],
                             func=Act.Exp)
        nc.scalar.activation(out=v_cols[:, ksl, :], in_=s_cols[:, ksl, :],
                             func=Act.Exp, scale=ALPHA)

    pa = pools["pa"]
    with tc.tile_pool(name="pha_ps", bufs=1, space="PSUM") as pap:
        w_sb = pa.tile([OUT_F, IN_F], f32, bufs=2, tag="w_sb")
        nc.sync.dma_start(out=w_sb, in_=w_d)
        # a as lhsT [64, 2]: col0 = a1, col1 = a2
        a_mat = pa.tile([OUT_F, 2], f32, bufs=2, tag="a_mat")
        nc.sync.dma_start(
            out=a_mat,
            in_=bass.AP(tensor=a_d.tensor, offset=0,
                        ap=[[1, OUT_F], [OUT_F, 2]]))

        # W.T tiles [k 128, f 64] via PE transpose, all 4 in one psum bank
        wt_all = pa.tile([128, NKC, OUT_F], f32, bufs=2, tag="wt_all")
        wt_ps = pap.tile([128, NKC, OUT_F], f32, bufs=1, tag="misc")
        for kc in range(NKC):
            nc.tensor.transpose(wt_ps[:, kc, :],
                                w_sb[:, kc * 128:(kc + 1) * 128],
                                ident[:OUT_F, :OUT_F])
        nc.scalar.copy(out=wt_all, in_=wt_ps)

        # combined vector c = W.T @ a [512, 2]: s = Wh@a == h@c, so s comes
        # straight from the h transposes -- no dependence on Wh at all.
        # scr is one psum bank shared (sequentially) by warmup, ct, cc, trA.
        scr = pap.tile([128, 512], f32, bufs=1, tag="scratch")
        # PE warm-up while the first h tiles are still in flight: ~3.4us of
        # activity flips the HAM clock gate to 2.4 GHz before real work
        for _wm in range(16):
            nc.tensor.transpose(scr[:, 128:256], ident, ident)
        nc.tensor.matmul(scr[0:2, :], lhsT=a_mat, rhs=w_sb,
                         start=True, stop=True)
        ct_sb = pa.tile([2, IN_F], f32, bufs=2, tag="ct_sb")
        nc.scalar.copy(out=ct_sb, in_=scr[0:2, :])
        for kc in range(NKC):
            nc.tensor.transpose(scr[:, 504 + 2 * kc:506 + 2 * kc],
                                ct_sb[:, kc * 128:(kc + 1) * 128],
                                ident[:2, :2])
        c_cols = pa.tile([128, NKC, 2], f32, bufs=2, tag="c_cols")
        nc.vector.tensor_copy(out=c_cols, in_=scr[:, 504:512])
        # all h loads issued up-front so no store ever head-of-line blocks
        # a later h prefetch on the in-order SP queue
        h_tiles = []
        for ic in range(NIC):
            h_tile = pa.tile([128, IN_F], f32, bufs=2 * NIC, tag="h_tile")
            nc.sync.dma_start(out=h_tile,
                              in_=h_d[ic * 128:(ic + 1) * 128, :])
            h_tiles.append(h_tile)
        for ic in range(NIC):
            h_tile = h_tiles[ic]
            # transpose all 4 k-chunks into one [128, 4, 128] psum bank
            ht_ps = pap.tile([128, NKC, 128], f32, bufs=2)
            for kc in range(NKC):
                nc.tensor.transpose(ht_ps[:, kc, :],
                                    h_tile[:, kc * 128:(kc + 1) * 128],
                                    ident)
            ht_sb = pa.tile([128, NKC, 128], f32, bufs=5, tag="ht_sb")
            nc.scalar.copy(out=ht_sb[:, 0:2, :], in_=ht_ps[:, 0:2, :])
            nc.vector.tensor_copy(out=ht_sb[:, 2:4, :], in_=ht_ps[:, 2:4, :])
            # s for this i-chunk, straight from ht: s[m, i] = sum_k c[k, m] h[i, k]
            sx_ps = pap.tile([2, 128], f32, bufs=1, tag="sx")
            for kc in range(NKC):
                nc.tensor.matmul(sx_ps, lhsT=c_cols[:, kc, :],
                                 rhs=ht_sb[:, kc, :],
                                 start=(kc == 0), stop=(kc == NKC - 1))
            nc.vector.tensor_copy(out=s_own_sb[:, ic * 128:(ic + 1) * 128],
                                  in_=sx_ps)
            wh_ps = pap.tile([128, OUT_F], f32, bufs=2)
            for kc in range(NKC):
                nc.tensor.matmul(wh_ps, lhsT=ht_sb[:, kc, :],
                                 rhs=wt_all[:, kc, :],
                                 start=(kc == 0), stop=(kc == NKC - 1))
            wh_sb = pa.tile([128, OUT_F], f32, bufs=4, tag="wh_sb")
            nc.scalar.copy(out=wh_sb, in_=wh_ps)
            nc.sync.dma_start(out=wh_own_dram[ic * 128:(ic + 1) * 128, :],
                              in_=wh_sb)
            # gather this i-chunk's Wh rows from all cores right away
            if sim_collectives:
                nc.gpsimd.dma_start(
                    out=bass.AP(tensor=whg_dram[ic].tensor, offset=0,
                                ap=[[128 * OUT_F, NCORES], [1, 128 * OUT_F]]),
                    in_=bass.AP(tensor=wh_own_dram.tensor,
                                offset=ic * 128 * OUT_F,
                                ap=[[0, NCORES], [1, 128 * OUT_F]]))
            else:
                nc.gpsimd.collective_compute(
                    "AllGather", mybir.AluOpType.bypass,
                    replica_groups=[list(range(NCORES))],
                    ins=[wh_own_dram[ic * 128:(ic + 1) * 128, :].opt()],
                    outs=[whg_dram[ic].opt()])
            # gather each s half the moment its last i-chunk's s is in SBUF
            if ic == 3 or ic == NIC - 1:
                gather_s_half(0 if ic == 3 else 1)
            if ic == 5:
                # half-0 s data has landed by now; load + prep it while the
                # tail of phase A still runs
                nc.scalar.dma_start(out=s_half_sb[0], in_=s_full_dram[0])
            if ic == NIC - 1:
                prep_s_half(0, scr[:, 0:64])

    # ------------- Phase C0: s1b, p/q --------
    with tc.tile_pool(name="phc_ps", bufs=1, space="PSUM") as pcp:
        # s1 of own rows broadcast across partitions as f16 [128, RPC],
        # via PE rank-1 matmul (neg_row x s1_row, negated on copy-out)
        s1b_ps0 = pcp.tile([128, 512], f32)
        s1b_ps1 = pcp.tile([128, 512], f32)
        nc.tensor.matmul(s1b_ps0, lhsT=neg_row, rhs=s_own_sb[0:1, 0:512],
                         start=True, stop=True)
        nc.tensor.matmul(s1b_ps1, lhsT=neg_row, rhs=s_own_sb[0:1, 512:1024],
                         start=True, stop=True)
        nc.scalar.activation(out=s1b[:, 0:512], in_=s1b_ps0,
                             func=Act.Copy, scale=-1.0)
        nc.vector.tensor_scalar(out=s1b[:, 512:1024], in0=s1b_ps1,
                                scalar1=-1.0, scalar2=None, op0=Alu.mult)

        # own s1 in per-partition columns: s1_cols[p, k] = s1[k*128+p]
        tr2_ps = pcp.tile([128, NIC, 2], f32)
        for k in range(NIC):
            nc.tensor.transpose(tr2_ps[:, k, :],
                                s_own_sb[:, k * 128:(k + 1) * 128],
                                ident[:2, :2])
        s1_cols = sc.tile([128, NIC, 2], f32, name=f"s1_cols_{rep}")
        nc.scalar.copy(out=s1_cols, in_=tr2_ps)

    p_cols = sc.tile([128, NIC, 2], f32, name=f"p_cols_{rep}")
    nc.scalar.activation(out=p_cols, in_=s1_cols, func=Act.Exp)
    q_cols = sc.tile([128, NIC, 2], f32, name=f"q_cols_{rep}")
    nc.scalar.activation(out=q_cols, in_=s1_cols, func=Act.Exp, scale=ALPHA)

    # ---------------- Phase D: masks as PE weights, accumulate [i, f] ----
    sv_sb = sc.tile([1, F1], f32, name=f"sv_sb_{rep}")
    # Sv accumulator, summed across j-chunks on the (otherwise idle) Pool
    # engine instead of 64 extra PE matmuls
    sv_acc = sc.tile([128, F1], f32, name=f"sv_acc_{rep}")
    nc.vector.memset(sv_acc, 0.0)
    pd, pw, pdm, pe = pools["pd"], pools["pw"], pools["pm"], pools["pe"]
    with tc.tile_pool(name="phd_ps", bufs=1, space="PSUM") as pdp:
        # 4 bank-aligned psum tiles, 2 accumulators each: [i, f] layout
        pair_ps = [pdp.tile([128, 2, 256], f32, name=f"pair{x}_{rep}")
                   for x in range(4)]
        sv_ps = pdp.tile([1, F1], f32, name=f"svp_{rep}")

        jidx = 0
        for k in range(NIC):
            if k == 1:
                # the half-1 gather has landed by now; loading it here (not
                # before k=0) keeps it off the k<4 critical path
                nc.scalar.dma_start(out=s_half_sb[1], in_=s_full_dram[1])
            if k == 2:
                # k>=4 s prep, deferred so its gather/DMA never head-of-line
                # blocks the k<4 work on the in-order queues
                with tc.tile_pool(name="phc2_ps", bufs=1,
                                  space="PSUM") as pcp2:
                    trB_ps = pcp2.tile([128, 64], f32)
                    prep_s_half(1, trB_ps)
            for hlf in range(2):
                whc4 = pd.tile([128, 4, F1], f32)
                nc.vector.memset(whc4[:, :, OUT_F:F1], 1.0)
                nc.sync.dma_start(
                    out=whc4[:, :, 0:OUT_F],
                    in_=bass.AP(tensor=whg_dram[k].tensor,
                                offset=hlf * 4 * 128 * OUT_F,
                                ap=[[OUT_F, 128], [128 * OUT_F, 4],
                                    [1, OUT_F]]))
                for g4 in range(4):
                    g = hlf * 4 + g4
                    jc = g * NIC + k
                    mask = pdm.tile([128, RPC], f16)
                    nc.vector.tensor_scalar(
                        out=mask, in0=s1b,
                        scalar1=s_cols[:, k, 2 * g + 1:2 * g + 2],
                        scalar2=0.0, op0=Alu.add, op1=Alu.is_ge)
                    whuv = pw.tile([128, FUV], f16)
                    nc.scalar.activation(out=whuv[:, 0:F1],
                                         in_=whc4[:, g4, :], func=Act.Copy,
                                         scale=u_cols[:, k, 2 * g + 1:2 * g + 2])
                    # v half also on Act (whu+whv ~560ns/jc stays under the
                    # PE pace; DVE keeps only mask + Sv-accumulate)
                    nc.scalar.activation(out=whuv[:, F1:FUV],
                                         in_=whc4[:, g4, :], func=Act.Copy,
                                         scale=v_cols[:, k, 2 * g + 1:2 * g + 2])
                    nc.vector.tensor_tensor(out=sv_acc, in0=sv_acc,
                                            in1=whuv[:, F1:FUV],
                                            op=Alu.add)
                    st = (jidx == 0)
                    sp = (jidx == NJC - 1)
                    for ic in range(NIC):
                        # one accumulation group per PSUM bank: only the
                        # bank's first MM starts it, only its last stops it
                        nc.tensor.matmul(
                            pair_ps[ic // 2][:, ic % 2, 0:FUV],
                            lhsT=mask[:, ic * 128:(ic + 1) * 128],
                            rhs=whuv, start=(st and ic % 2 == 0),
                            stop=(sp and ic % 2 == 1))
                    jidx += 1

        # ---------------- Phase E: combine, divide, store ----------------
        # out[i, f] = p_i*Du[i, f] - q_i*(Dv[i, f] - Sv[f]); den = col 64.
        # Fold -Sv into the v-half of each accumulator with a rank-1 matmul.
        nc.tensor.matmul(sv_ps, lhsT=ones_col32, rhs=sv_acc,
                         start=True, stop=True)
        nc.scalar.copy(out=sv_sb, in_=sv_ps)
        out_sb = sc.tile([128, NIC, OUT_F], f32, name=f"out_sb_{rep}")
        # all 8 rank-1 fixups first: releases PE for the next rep's phase A
        for ic in range(NIC):
            pv = pair_ps[ic // 2][:, ic % 2, :]
            nc.tensor.matmul(pv[:, F1:FUV], lhsT=neg_row,
                             rhs=sv_sb, start=False, stop=True,
                             skip_group_check=True)
        for ic in range(NIC):
            pv = pair_ps[ic // 2][:, ic % 2, :]
            r1 = pe.tile([128, F1], f32, bufs=8)
            nc.scalar.activation(out=r1, in_=pv[:, 0:F1], func=Act.Copy,
                                 scale=p_cols[:, ic, 0:1])
            r2 = pe.tile([128, F1], f32, bufs=8)
            nc.scalar.activation(out=r2, in_=pv[:, F1:FUV], func=Act.Copy,
                                 scale=q_cols[:, ic, 0:1])
            r4 = pe.tile([128, F1], f32, bufs=8)
            nc.vector.tensor_tensor(out=r4, in0=r1, in1=r2, op=Alu.subtract)
            rec = pe.tile([128, 1], f32, bufs=8)
            nc.vector.reciprocal(out=rec, in_=r4[:, OUT_F:F1])
            nc.vector.tensor_scalar(out=out_sb[:, ic, :], in0=r4[:, 0:OUT_F],
                                    scalar1=rec, scalar2=None, op0=Alu.mult)
        # single batched store: out_d[ic*128 + p, f] = out_sb[p, ic, f]
        nc.scalar.dma_start(
            out=bass.AP(tensor=out_d.tensor, offset=0,
                        ap=[[OUT_F, 128], [128 * OUT_F, NIC], [1, OUT_F]]),
            in_=out_sb)


def _get_nc(unroll=1):
    key = ("nc", unroll)
    if key not in _CACHE:
        _CACHE[key] = _build_kernel(unroll)
    return _CACHE[key]


def kernel(h, adj, W, a, _unroll=1, _return_raw=False):
    from concourse.bass_utils import run_bass_kernel_spmd

    nc = _get_nc(_unroll)
    h = np.ascontiguousarray(np.asarray(h, dtype=np.float32))
    W = np.ascontiguousarray(np.asarray(W, dtype=np.float32))
    a = np.ascontiguousarray(np.asarray(a, dtype=np.float32))
    in_maps = [
        {"h_shard": h[c * RPC:(c + 1) * RPC], "w_in": W, "a_in": a}
        for c in range(NCORES)
    ]
    res = run_bass_kernel_spmd(nc, in_maps, list(range(NCORES)))
    out = np.concatenate([res.results[c]["out_shard"] for c in range(NCORES)],
                         axis=0)
    if _return_raw:
        return out, res
    return out
